# revision 18
# baseline (speedup 1.0000x reference)
"""ExpertLoRA MoE kernel for 8x TRN2 NeuronCores (expert-parallel, routed).

Strategy
--------
The reference computes all 16 experts densely over all 1024 tokens and then
masks with the routing weights.  Only top-2 experts per token actually
contribute, so we:

  * host: fold LoRA into the main weights (W_eff = W + A@B*scaling — exact),
    de-interleave gate/up columns, compute per-expert routed token lists
    (weights of duplicate slots summed), sort experts by load and assign the
    8 busiest to core slot 0 (capacity C0) and the 8 lightest to slot 1
    (capacity C1 <= C0), gather + transpose tokens per expert, and pack all
    weights transfer-major so every device DMA reads one contiguous
    DRAM range.
  * weights ship as fp8e3 (e3m4) scaled by WSCALE=64: a global power-of-2
    scale keeps full fp8 relative precision (~1.8% rms/element) while
    halving HBM traffic vs fp16.  The device compensates exactly:
    x is pre-divided by 64 on the host (fp16, power-of-2 => lossless), so
    gate/up psums come out true-scale; the up path multiplies by 1/64 so
    g feeds the (64x) down weights and the down psum is true y.
  * device (SPMD over 8 cores, 2 experts each): transposed-layout expert MLP
      guT = Wg^T @ xT ; upT = Wu^T @ xT     (PE, e3 weights x fp16 acts)
      gT  = act(guT, upT)                    (ACT + DVE)
      yT  = Wd^T @ gT                        (PE), cast fp16, DMA out
    All weights are SBUF-resident; every weight DMA is issued up-front on the
    nc.sync HWDGE queue in consumption order (the ACT queue must stay
    DMA-free or activations stall behind DMA issues).  The first gate/up
    block is split into quarters so the PE can start ~0.5us earlier.
    ~3us of dummy matmuls bridge the PE through the HAM cold window while
    the first blocks stream.  When the host proves |gu| << 7 on the routed
    tokens (it is, ~3.9 max), the +-7 clips are dead code and the
    activation chain collapses to one ACT op (gelu-sigmoid with fused
    bias) + one DVE add-mul + one DVE mul.
  * host: scatter-add  out[tok] += w * (y + bias)  per expert, plus an exact
    numpy fallback for the (practically impossible) case of an expert
    exceeding capacity.

Accuracy: fp8e3 weights with fp16 activations / fp32 accumulation give
1.48e-2 relative absmax error end-to-end (measured on HW, matches the
numpy simulation exactly; gate is 2e-2).
"""
import numpy as np

E, H, F, R = 16, 1024, 1024, 16
D = 2 * F
TOPK = 2
SCALING = 16.0 / R
LIMIT = 7.0
ACT_ALPHA = 1.702
B_, S_ = 2, 512
T = B_ * S_
N_CORES = 8
EPC = E // N_CORES        # experts per core
KH = H // 128             # contraction tiles for H
KF = F // 128             # contraction tiles for F
MF = F // 128             # output tiles for F (gate or up half)
MH = H // 128             # output tiles for H
MP = MF // 2              # gate/up m-pairs per weight DMA block
HQ = MH // 4              # down h-quads per weight DMA block
C_MAX = 512               # hard cap (one PSUM bank); overflow -> host fallback
N_WARM = 6                # dummy matmuls bridging the PE to the first block
                          # (~2.6us cold: covers the DMA first-block latency;
                          # the HAM un-throttle lags 4-10us after PE start
                          # regardless, so longer warm-up is pure overhead)

# ---- pacing model for filler placement (ns) -------------------------------
SIM_RATE = 368.0          # DMA bytes/ns: sized BELOW the ~390-410 typical so
                          # fillers still cover PE gaps when the stream is
                          # slow (an underfilled gap risks a 2-5us HAM
                          # re-throttle; overfill just absorbs PE slack)
SIM_DMA0 = 1400           # first-byte latency after user code starts
SIM_MM = 75               # effective per-matmul pace (N~144, warm)
SIM_FILL = 213            # one N=512 filler matmul
SIM_WARM_MM = 427         # cold N=512 warmup matmul

DT_NAME = "float16"
WSCALE = 64.0             # weights stored as fp8e3(WSCALE * W); x fed as
                          # x/WSCALE so gate/up psums come out true-scale,
                          # and the up path folds 1/WSCALE into g so the
                          # down psum (fp8e3 weights again) is true y

_CACHE = {}


def _np_dt():
    import ml_dtypes
    return {"float16": np.float16, "float32r": np.float32,
            "float32": np.float32, "bfloat16": ml_dtypes.bfloat16}[DT_NAME]


def _np_w_dt():
    import ml_dtypes
    return ml_dtypes.float8_e3m4


def _build_nc(CS, fast_act=False):
    """Build the SPMD per-core Bass program (same NEFF for all 8 cores).

    CS: per-slot token capacities (C0, C1), C0 >= C1.
    fast_act: drop the +-7 clips (host proved |gu| << 7 on this data).
    """
    import concourse.bass as bass
    import concourse.tile as tile
    import concourse.mybir as mybir
    from concourse import bacc

    DT = getattr(mybir.dt, DT_NAME)
    WDT = mybir.dt.float8e3
    WB = 1                    # weight bytes per element
    f32 = mybir.dt.float32
    AF = mybir.ActivationFunctionType
    OP = mybir.AluOpType

    nc = bacc.Bacc("TRN2", target_bir_lowering=False, debug=False,
                   enable_asserts=False, num_devices=N_CORES)

    # (e, mp, p, mi, gu, k, j): gate/up weights; one whole-block DMA per
    # (e, mp) reads a single contiguous DRAM range (best HBM locality)
    wgu_d = nc.dram_tensor("wgu", [EPC, MP, 128, 2, 2, KH, 128], WDT,
                           kind="ExternalInput").ap()
    # (e, hq, p, hi, k, j): down weights, one contiguous DMA per (e, hq)
    wd_d = nc.dram_tensor("wd", [EPC, HQ, 128, 4, KF, 128], WDT,
                          kind="ExternalInput").ap()
    xt_ds = [nc.dram_tensor(f"xt{j}", [128, KH, CS[j]], DT,
                            kind="ExternalInput").ap() for j in range(EPC)]
    # (p, e, which, m): which 0=gate bias, 1=up bias (+1 folded)
    bz_d = nc.dram_tensor("bz", [128, EPC, 2, 8], f32, kind="ExternalInput").ap()
    # (hpair, p, hi, t): outputs, contiguous per 2-h-tile DMA
    yt_ds = [nc.dram_tensor(f"yt{j}", [MH // 2, 128, 2, CS[j]], DT,
                            kind="ExternalOutput").ap() for j in range(EPC)]
    dbg_d = nc.dram_tensor("dbg", [128, 8], f32, kind="ExternalOutput").ap()

    with tile.TileContext(nc) as tc:
        with tc.tile_pool(name="const", bufs=1) as const, \
             tc.tile_pool(name="wres", bufs=1) as wres, \
             tc.tile_pool(name="g", bufs=2) as gpool, \
             tc.tile_pool(name="act", bufs=4) as apool, \
             tc.tile_pool(name="y", bufs=2) as ypool, \
             tc.tile_pool(name="ps", bufs=7, space="PSUM") as pspool, \
             tc.tile_pool(name="wps", bufs=1, space="PSUM") as wpspool:

            # ---- all input DMAs up-front, in consumption order ----------
            # Everything goes on the nc.sync (SP) HWDGE queue: the ACT
            # engine's queue must stay free of DMA issues, or activations
            # queue up behind slow DMA_DIRECT2D instructions (FIFO engine
            # queues).  A single queue still uses all 16 SDMA engines.
            xt_sbs = []
            for j in range(EPC):
                xt_sb_j = const.tile([128, KH, CS[j]], DT, tag=f"xt{j}")
                xt_sbs.append(xt_sb_j)
            bz_sb = const.tile([128, EPC, 2, 8], f32)
            nc.sync.dma_start(xt_sbs[0][:], xt_ds[0])
            cum = [128 * KH * CS[0] * 2]

            arrive = {}

            def track(key, nbytes):
                cum[0] += nbytes
                arrive[key] = SIM_DMA0 + cum[0] / SIM_RATE

            wgu_t, wd_t = {}, {}
            for e in range(EPC):
                for mp in range(MP):
                    t = wres.tile([128, 2, 2, KH, 128], WDT, tag=f"wg{e}_{mp}")
                    if e == 0 and mp == 0:
                        # split the first block per (mi, gu) quarter so the
                        # PE can start on (mi=0, gu=0) ~0.5us earlier
                        for mi in range(2):
                            for g in range(2):
                                nc.sync.dma_start(t[:, mi, g],
                                                  wgu_d[e, mp, :, mi, g])
                        track(("gu", e, mp), 128 * 4 * KH * 128 * WB)
                        arrive[("gu", e, mp)] = (SIM_DMA0 +
                                                 (cum[0] - 3 * KH * 128 * 128
                                                  * WB) / SIM_RATE)
                    else:
                        # whole-block DMA: 4KB/partition contiguous=peak rate
                        nc.sync.dma_start(t[:], wgu_d[e, mp])
                        track(("gu", e, mp), 128 * 4 * KH * 128 * WB)
                    wgu_t[(e, mp)] = t
                    if e == 0 and mp == 0:   # tiny bias DMA after first block
                        nc.sync.dma_start(bz_sb[:], bz_d)
                        cum[0] += 128 * EPC * 2 * 8 * 4
                for hq in range(HQ):
                    t = wres.tile([128, 4, KF, 128], WDT, tag=f"wd{e}_{hq}")
                    nc.sync.dma_start(t[:], wd_d[e, hq])
                    track(("d", e, hq), 128 * 4 * KF * 128 * WB)
                    arrive[("d", e, hq, 0)] = arrive[("d", e, hq)]
                    arrive[("d", e, hq, 1)] = arrive[("d", e, hq)]
                    wd_t[(e, hq)] = t
                if e == 0:    # xt for expert 1 after all of e0's weights
                    nc.sync.dma_start(xt_sbs[1][:], xt_ds[1])
                    cum[0] += 128 * KH * CS[1] * 2

            # ---- filler plan: simulate PE vs DMA arrivals ---------------
            units = []
            for e in range(EPC):
                for mp in range(MP):
                    units.append(("gu", e, mp))
                for hq in range(HQ):
                    for half in range(2):
                        units.append(("d", e, hq, half))
            fillers = {}
            t_pe = 900 + N_WARM * SIM_WARM_MM
            for ui, key in enumerate(units):
                mm_t = SIM_MM * CS[key[1]] / 144.0
                n_mm = 32 if key[0] == "gu" else 16
                gap = arrive[key] - t_pe
                n_fill = 0
                if gap > 200 and ui < len(units) - 2:
                    n_fill = min(10, int(0.9 * gap / SIM_FILL))
                fillers[key] = n_fill
                t_pe += n_fill * SIM_FILL
                t_pe = max(t_pe, arrive[key]) + n_mm * mm_t

            # ---- ACT table preload: tables load off the critical path
            # (reads uninitialized SBUF on purpose -- zero dependencies)
            wsink = const.tile([128, 8], f32)
            if not fast_act:
                nc.scalar.activation(wsink[:], wsink[:], AF.Identity)
            nc.scalar.activation(wsink[:], wsink[:], AF.Gelu_apprx_sigmoid)

            # ---- PE warm-up: dummy matmuls while the first blocks stream --
            # (wz memset runs on GpSimd, whose queue is free ~1.5us before
            # the DVE finishes its preamble -- warm-up starts that much
            # earlier)
            wz = const.tile([128, 512], DT)
            nc.gpsimd.memset(wz[:], 0.0)
            warm_ps = wpspool.tile([128, 512], f32)
            for i in range(N_WARM):
                nc.tensor.matmul(warm_ps[:], wz[:, 0:128], wz[:],
                                 start=(i == 0), stop=(i == N_WARM - 1))
            nc.vector.tensor_copy(wsink[:], warm_ps[:, 0:8])
            nc.sync.dma_start(dbg_d, wsink[:])

            # ---- main expert loop ---------------------------------------
            def pe_filler(n):
                # dummy matmuls pace the PE to the DMA stream so it never
                # idles long enough for the HAM clock-gate to re-throttle
                for _ in range(n):
                    nc.tensor.matmul(warm_ps[:], wz[:, 0:128], wz[:],
                                     start=True, stop=True)

            for e in range(EPC):
                C = CS[e]
                xt_sb = xt_sbs[e]
                gT = gpool.tile([128, KF, C], DT, tag="gT")
                for mp in range(MP):
                    wgut = wgu_t[(e, mp)]
                    for mi in range(2):
                        m = 2 * mp + mi
                        if mi == 0:
                            pe_filler(fillers[("gu", e, mp)])
                        psg = pspool.tile([128, C], f32, tag="ps")
                        psu = pspool.tile([128, C], f32, tag="ps")
                        for k in range(KH):
                            nc.tensor.matmul(psg[:], wgut[:, mi, 0, k],
                                             xt_sb[:, k],
                                             start=(k == 0), stop=(k == KH - 1))
                        for k in range(KH):
                            nc.tensor.matmul(psu[:], wgut[:, mi, 1, k],
                                             xt_sb[:, k],
                                             start=(k == 0), stop=(k == KH - 1))
                        if fast_act:
                            # glu = gelusig(psg + bg)      (clip provably dead)
                            glu = apool.tile([128, C], f32, tag="glu")
                            nc.scalar.activation(glu[:], psg[:],
                                                 AF.Gelu_apprx_sigmoid,
                                                 bias=bz_sb[:, e, 0, m:m + 1])
                            # up1/WS = (psu + (bu+1)) / WSCALE  (so g feeds
                            # the fp8e3(WSCALE*Wd) down matmul at true scale)
                            up = apool.tile([128, C], f32, tag="up")
                            nc.vector.tensor_scalar(up[:], psu[:],
                                                    bz_sb[:, e, 1, m:m + 1],
                                                    1.0 / WSCALE,
                                                    OP.add, OP.mult)
                        else:
                            # gate = min(psg + bg, 7)
                            gate = apool.tile([128, C], f32, tag="gate")
                            nc.vector.tensor_scalar(gate[:], psg[:],
                                                    bz_sb[:, e, 0, m:m + 1],
                                                    LIMIT, OP.add, OP.min)
                            # glu = gate * sigmoid(1.702 * gate)  (one ACT op)
                            glu = apool.tile([128, C], f32, tag="glu")
                            nc.scalar.activation(glu[:], gate[:],
                                                 AF.Gelu_apprx_sigmoid)
                            # (clip(psu + bu, -7, 7) + 1) / WSCALE  (bu has +1
                            #   folded pre-clip-shift: ACT adds bias + scales,
                            #   DVE clips in one op)
                            upb = apool.tile([128, C], f32, tag="upb")
                            nc.scalar.activation(upb[:], psu[:], AF.Identity,
                                                 bias=bz_sb[:, e, 1, m:m + 1])
                            upc = apool.tile([128, C], f32, tag="upc")
                            nc.vector.tensor_scalar(upc[:], upb[:],
                                                    LIMIT + 1.0,
                                                    -(LIMIT - 1.0),
                                                    OP.min, OP.max)
                            up = apool.tile([128, C], f32, tag="up")
                            nc.vector.tensor_scalar_mul(up[:], upc[:],
                                                        1.0 / WSCALE)
                        # gT[:, m] = up1 * glu   (cast to DT)
                        nc.vector.tensor_mul(out=gT[:, m], in0=up[:], in1=glu[:])
                yst = ypool.tile([128, MH, C], DT, tag="y")
                for hq in range(HQ):
                    wdt = wd_t[(e, hq)]
                    for hi in range(4):
                        h = 4 * hq + hi
                        if hi % 2 == 0:
                            pe_filler(fillers[("d", e, hq, hi // 2)])
                        psy = pspool.tile([128, C], f32, tag="ps")
                        for k in range(KF):
                            nc.tensor.matmul(psy[:], wdt[:, hi, k], gT[:, k],
                                             start=(k == 0), stop=(k == KF - 1))
                        nc.vector.tensor_copy(yst[:, h], psy[:])
                        if hi % 2 == 1:
                            h0 = h - 1
                            nc.sync.dma_start(yt_ds[e][h0 // 2],
                                              yst[:, h0:h0 + 2])
    nc.compile()
    return nc


def _get_nc(CS, fast_act):
    key = ("nc", CS, fast_act)
    if key not in _CACHE:
        _CACHE[key] = _build_nc(CS, fast_act)
    return _CACHE[key]


def _route(router_indices, routing_weights):
    """Per-expert unique token list + summed weights."""
    ri = np.asarray(router_indices)
    rw = np.asarray(routing_weights, dtype=np.float32)
    idxs, ws = [], []
    for e in range(E):
        m = ri == e
        any_m = m.any(axis=1)
        idx = np.nonzero(any_m)[0]
        w = (rw * m).sum(axis=1)[idx]
        idxs.append(idx.astype(np.int64))
        ws.append(w)
    return idxs, ws


def _fold_weights(gate_up_proj, gate_up_bias, down_proj, down_bias,
                  lora_gate_up_A, lora_gate_up_B, lora_down_A, lora_down_B,
                  perm):
    """LoRA-folded, gate/up-split, transfer-major packed per-core tensors.

    perm: [N_CORES][EPC] expert index assigned to each (core, slot).
    """
    w_dt = _np_w_dt()
    gup = np.asarray(gate_up_proj, dtype=np.float32)
    gub = np.asarray(gate_up_bias, dtype=np.float32)
    dwn = np.asarray(down_proj, dtype=np.float32)
    Agu = np.asarray(lora_gate_up_A, dtype=np.float32)
    Bgu = np.asarray(lora_gate_up_B, dtype=np.float32)
    Ad = np.asarray(lora_down_A, dtype=np.float32)
    Bd = np.asarray(lora_down_B, dtype=np.float32)

    # W_eff = W + A @ B * s    (batched over experts)
    wgu = gup + np.einsum("ehr,erd->ehd", Agu, Bgu) * SCALING     # [E, H, D]
    wdn = dwn + np.einsum("efr,erh->efh", Ad, Bd) * SCALING       # [E, F, H]

    wg = wgu[:, :, 0::2]                                          # [E, H, F]
    wu = wgu[:, :, 1::2]
    bgs = gub[:, 0::2]                                            # [E, F]
    bus = gub[:, 1::2] + 1.0                                      # fold (+1)

    def prep(w):
        # [E, K*128, M*128] -> [E, k, p, m, j] -> [E, p, m, k, j]
        w = w.reshape(E, KH, 128, MF, 128).transpose(0, 2, 3, 1, 4)
        return w
    wgp = prep(wg).reshape(E, 128, MP, 2, KH, 128)
    wup = prep(wu).reshape(E, 128, MP, 2, KH, 128)
    wgu_all = np.stack([wgp, wup], axis=4)  # [E, 128, MP, mi, gu, k, j]
    wdp = wdn.reshape(E, KF, 128, MH, 128).transpose(0, 2, 3, 1, 4)
    wdp = wdp.reshape(E, 128, HQ, 4, KF, 128)

    # biases: [E, 128, 2, 8]
    bz = np.stack([
        bgs.reshape(E, MF, 128).transpose(0, 2, 1),
        bus.reshape(E, MF, 128).transpose(0, 2, 1),
    ], axis=2)

    wgu_cores, wd_cores, bz_cores = [], [], []
    for c in range(N_CORES):
        sel = list(perm[c])
        wgu_cores.append(
            (wgu_all[sel].transpose(0, 2, 1, 3, 4, 5, 6)
             * WSCALE).astype(w_dt))
        wd_cores.append(
            (wdp[sel].transpose(0, 2, 1, 3, 4, 5) * WSCALE).astype(w_dt))
        bz_cores.append(np.ascontiguousarray(
            bz[sel].transpose(1, 0, 2, 3), dtype=np.float32))
    return {"wgu": wgu_cores, "wd": wd_cores, "bz": bz_cores,
            "wgu_eff": wgu}


def _expert_mlp_exact(x_e, Wg, Wu, bg, bu, Wd, bd):
    """fp32 numpy fallback (host) for capacity-overflow tokens."""
    gate = np.minimum(x_e @ Wg + bg, LIMIT)
    up = np.clip(x_e @ Wu + bu, -LIMIT, LIMIT)
    glu = gate / (1.0 + np.exp(-gate * ACT_ALPHA))
    g = (up + 1.0) * glu
    return g @ Wd + bd


def _host_expert(x, idx, e, gate_up_proj, gate_up_bias, down_proj, down_bias,
                 lora_gate_up_A, lora_gate_up_B, lora_down_A, lora_down_B):
    gup = np.asarray(gate_up_proj[e], dtype=np.float32)
    Agu = np.asarray(lora_gate_up_A[e], dtype=np.float32)
    Bgu = np.asarray(lora_gate_up_B[e], dtype=np.float32)
    wgu = gup + Agu @ Bgu * SCALING
    dwn = np.asarray(down_proj[e], dtype=np.float32)
    Ad = np.asarray(lora_down_A[e], dtype=np.float32)
    Bd = np.asarray(lora_down_B[e], dtype=np.float32)
    wdn = dwn + Ad @ Bd * SCALING
    gub = np.asarray(gate_up_bias[e], dtype=np.float32)
    return _expert_mlp_exact(x[idx], wgu[:, 0::2], wgu[:, 1::2],
                             gub[0::2], gub[1::2], wdn,
                             np.asarray(down_bias[e], dtype=np.float32))


def kernel(hidden_states, router_indices, routing_weights,
           gate_up_proj, gate_up_bias, down_proj, down_bias,
           lora_gate_up_A, lora_gate_up_B, lora_down_A, lora_down_B):
    from concourse import bass_utils

    np_dt = _np_dt()
    x = np.asarray(hidden_states, dtype=np.float32).reshape(T, H)
    idxs, ws = _route(router_indices, routing_weights)

    # sort experts by load (balances per-core totals); slot capacities track
    # the max load in each half so light slots do less dead work
    order = sorted(range(E), key=lambda e: -len(idxs[e]))
    perm = [[order[c], order[E - 1 - c]] for c in range(N_CORES)]
    C0 = min(C_MAX, max(8, -(-len(idxs[order[0]]) // 4) * 4))
    C1 = min(C0, max(8, -(-len(idxs[order[N_CORES]]) // 4) * 4))
    CS = (C0, C1)

    packed = _fold_weights(gate_up_proj, gate_up_bias, down_proj, down_bias,
                           lora_gate_up_A, lora_gate_up_B,
                           lora_down_A, lora_down_B, perm)

    # fast-act legality: the reference clips gate/up at +-7; if the actual
    # pre-activation magnitudes stay far below (host fp32 check on the real
    # routed tokens), the clips are dead code and the device can skip them.
    wgu_eff = packed["wgu_eff"]       # [E, H, D] fp32, LoRA folded
    gub = np.asarray(gate_up_bias, dtype=np.float32)
    gu_max = 0.0
    for e in range(E):
        if len(idxs[e]) == 0:
            continue
        gu = x[idxs[e][:C_MAX]] @ wgu_eff[e] + gub[e]
        gu_max = max(gu_max, float(np.abs(gu).max()))
    fast_act = gu_max < 6.0

    # gather + transpose tokens per expert: xt{j} [128, KH, CS[j]]
    in_maps = []
    for c in range(N_CORES):
        m = {"wgu": packed["wgu"][c], "wd": packed["wd"][c],
             "bz": packed["bz"][c]}
        for j in range(EPC):
            e = perm[c][j]
            Cj = CS[j]
            xt = np.zeros((128, KH, Cj), dtype=np_dt)
            idx = idxs[e][:Cj]
            if len(idx):
                # x[idx]/WSCALE : [n, H] -> T -> [KH, 128, n] -> [128, KH, n]
                xg = (x[idx].T.reshape(KH, 128, len(idx))
                      .transpose(1, 0, 2)) * (1.0 / WSCALE)
                xt[:, :, :len(idx)] = xg.astype(np_dt)
            m[f"xt{j}"] = xt
        in_maps.append(m)

    res = None
    try:
        nc = _get_nc(CS, fast_act)
        res = bass_utils.run_bass_kernel_spmd(
            nc, in_maps, core_ids=list(range(N_CORES)),
            **_CACHE.get("run_kwargs", {}))
    except Exception:
        try:
            nc = _get_nc(CS, fast_act)
            res = bass_utils.run_bass_kernel_spmd(
                nc, in_maps, core_ids=list(range(N_CORES)),
                **_CACHE.get("run_kwargs", {}))
        except Exception:
            res = None
    _CACHE["last_results"] = res
    if res is None:
        # device path failed: exact fp32 host fallback (slow but correct)
        out = np.zeros((T, H), dtype=np.float32)
        for e in range(E):
            idx = idxs[e]
            if not len(idx):
                continue
            y = _host_expert(x, idx, e, gate_up_proj, gate_up_bias,
                             down_proj, down_bias, lora_gate_up_A,
                             lora_gate_up_B, lora_down_A, lora_down_B)
            out[idx] += ws[e][:, None] * y
        return out.reshape(B_, S_, H)

    out = np.zeros((T, H), dtype=np.float32)
    for c in range(N_CORES):
        for j in range(EPC):
            e = perm[c][j]
            Cj = CS[j]
            yt = res.results[c][f"yt{j}"]      # [MH//2, 128, 2, Cj] fp16
            idx = idxs[e]
            n = min(len(idx), Cj)
            if n:
                # yt[hp, p, hi, t] -> y[t, (2hp+hi)*128+p]  (+ bias, host)
                y = np.ascontiguousarray(
                    yt[:, :, :, :n].transpose(3, 0, 2, 1)).reshape(n, H)
                y = y.astype(np.float32) + np.asarray(down_bias[e],
                                                      dtype=np.float32)
                out[idx[:n]] += ws[e][:n, None] * y
            if len(idx) > Cj:     # capacity overflow: exact host fallback
                ovf = idx[Cj:]
                y2 = _host_expert(x, ovf, e, gate_up_proj, gate_up_bias,
                                  down_proj, down_bias, lora_gate_up_A,
                                  lora_gate_up_B, lora_down_A, lora_down_B)
                out[ovf] += ws[e][Cj:, None] * y2
    return out.reshape(B_, S_, H)



# revision 20
# speedup vs baseline: 1.2416x; 1.2416x over previous
"""ExpertLoRA MoE kernel for 8x TRN2 NeuronCores (expert-parallel, routed).

Strategy
--------
The reference computes all 16 experts densely over all 1024 tokens and then
masks with the routing weights.  Only top-2 experts per token actually
contribute, so we:

  * host: fold LoRA into the main weights (W_eff = W + A@B*scaling — exact),
    de-interleave gate/up columns, compute per-expert routed token lists
    (weights of duplicate slots summed), sort experts by load and assign the
    8 busiest to core slot 0 (capacity C0) and the 8 lightest to slot 1
    (capacity C1 <= C0), gather + transpose tokens per expert, and pack all
    weights transfer-major so every device DMA reads one contiguous
    DRAM range.
  * weights ship as fp8e3 (e3m4) scaled by WSCALE=64: a global power-of-2
    scale keeps full fp8 relative precision (~1.8% rms/element) while
    halving HBM traffic vs fp16.  The device compensates exactly:
    x is pre-divided by 64 on the host (fp16, power-of-2 => lossless), so
    gate/up psums come out true-scale; the up path multiplies by 1/64 so
    g feeds the (64x) down weights and the down psum is true y.
  * device (SPMD over 8 cores, 2 experts each): transposed-layout expert MLP
      guT = Wg^T @ xT ; upT = Wu^T @ xT     (PE, e3 weights x fp16 acts)
      gT  = act(guT, upT)                    (ACT + DVE)
      yT  = Wd^T @ gT                        (PE), cast fp16, DMA out
    All weights are SBUF-resident; every weight DMA is issued up-front on the
    nc.sync HWDGE queue in consumption order (the ACT queue must stay
    DMA-free or activations stall behind DMA issues).  The first gate/up
    block is split into quarters so the PE can start ~0.5us earlier.
    ~3us of dummy matmuls bridge the PE through the HAM cold window while
    the first blocks stream.  When the host proves |gu| << 7 on the routed
    tokens (it is, ~3.9 max), the +-7 clips are dead code and the
    activation chain collapses to one ACT op (gelu-sigmoid with fused
    bias) + one DVE add-mul + one DVE mul.
  * host: scatter-add  out[tok] += w * (y + bias)  per expert, plus an exact
    numpy fallback for the (practically impossible) case of an expert
    exceeding capacity.

Accuracy: fp8e3 weights with fp16 activations / fp32 accumulation give
1.48e-2 relative absmax error end-to-end (measured on HW, matches the
numpy simulation exactly; gate is 2e-2).
"""
import numpy as np

E, H, F, R = 16, 1024, 1024, 16
D = 2 * F
TOPK = 2
SCALING = 16.0 / R
LIMIT = 7.0
ACT_ALPHA = 1.702
B_, S_ = 2, 512
T = B_ * S_
N_CORES = 8
EPC = E // N_CORES        # experts per core
KH = H // 128             # contraction tiles for H
KF = F // 128             # contraction tiles for F
MF = F // 128             # output tiles for F (gate or up half)
MH = H // 128             # output tiles for H
MP = MF // 2              # gate/up m-pairs per weight DMA block
HQ = MH // 4              # down h-quads per weight DMA block
C_MAX = 512               # hard cap (one PSUM bank); overflow -> host fallback
N_WARM = 6                # dummy matmuls bridging the PE to the first block
                          # (~2.6us cold: covers the DMA first-block latency;
                          # the HAM un-throttle lags 4-10us after PE start
                          # regardless, so longer warm-up is pure overhead)

# ---- pacing model for filler placement (ns) -------------------------------
SIM_RATE = 368.0          # DMA bytes/ns: sized BELOW the ~390-410 typical so
                          # fillers still cover PE gaps when the stream is
                          # slow (an underfilled gap risks a 2-5us HAM
                          # re-throttle; overfill just absorbs PE slack)
SIM_DMA0 = 1400           # first-byte latency after user code starts
SIM_MM = 75               # effective per-matmul pace (N~144, warm)
SIM_FILL = 213            # one N=512 filler matmul
SIM_WARM_MM = 427         # cold N=512 warmup matmul

DT_NAME = "float16"
WSCALE = 64.0             # weights stored as fp8e3(WSCALE * W); x fed as
                          # x/WSCALE so gate/up psums come out true-scale,
                          # and the up path folds 1/WSCALE into g so the
                          # down psum (fp8e3 weights again) is true y

_CACHE = {}


def _np_dt():
    import ml_dtypes
    return {"float16": np.float16, "float32r": np.float32,
            "float32": np.float32, "bfloat16": ml_dtypes.bfloat16}[DT_NAME]


def _np_w_dt():
    import ml_dtypes
    return ml_dtypes.float8_e3m4


def _build_nc(CS, fast_act=False):
    """Build the SPMD per-core Bass program (same NEFF for all 8 cores).

    CS: per-slot token capacities (C0, C1), C0 >= C1.
    fast_act: drop the +-7 clips (host proved |gu| << 7 on this data).
    """
    import concourse.bass as bass
    import concourse.tile as tile
    import concourse.mybir as mybir
    from concourse import bacc

    DT = getattr(mybir.dt, DT_NAME)
    WDT = mybir.dt.float8e3
    WB = 1                    # weight bytes per element
    f32 = mybir.dt.float32
    AF = mybir.ActivationFunctionType
    OP = mybir.AluOpType

    nc = bacc.Bacc("TRN2", target_bir_lowering=False, debug=False,
                   enable_asserts=False, num_devices=N_CORES)

    # (e, mp, p, mi, gu, k, j): gate/up weights; one whole-block DMA per
    # (e, mp) reads a single contiguous DRAM range (best HBM locality)
    wgu_d = nc.dram_tensor("wgu", [EPC, MP, 128, 2, 2, KH, 128], WDT,
                           kind="ExternalInput").ap()
    # (e, hq, p, hi, k, j): down weights, one contiguous DMA per (e, hq)
    wd_d = nc.dram_tensor("wd", [EPC, HQ, 128, 4, KF, 128], WDT,
                          kind="ExternalInput").ap()
    xt_ds = [nc.dram_tensor(f"xt{j}", [128, KH, CS[j]], DT,
                            kind="ExternalInput").ap() for j in range(EPC)]
    # (p, e, which, m): which 0=gate bias, 1=up bias (+1 folded)
    bz_d = nc.dram_tensor("bz", [128, EPC, 2, 8], f32, kind="ExternalInput").ap()
    # (hpair, p, hi, t): outputs, contiguous per 2-h-tile DMA
    yt_ds = [nc.dram_tensor(f"yt{j}", [MH // 2, 128, 2, CS[j]], DT,
                            kind="ExternalOutput").ap() for j in range(EPC)]
    dbg_d = nc.dram_tensor("dbg", [128, 8], f32, kind="ExternalOutput").ap()

    with tile.TileContext(nc) as tc:
        with tc.tile_pool(name="const", bufs=1) as const, \
             tc.tile_pool(name="wres", bufs=1) as wres, \
             tc.tile_pool(name="g", bufs=2) as gpool, \
             tc.tile_pool(name="act", bufs=4) as apool, \
             tc.tile_pool(name="y", bufs=2) as ypool, \
             tc.tile_pool(name="ps", bufs=7, space="PSUM") as pspool, \
             tc.tile_pool(name="wps", bufs=1, space="PSUM") as wpspool:

            # ---- all input DMAs up-front, in consumption order ----------
            # Weights stream on the nc.sync (SP) HWDGE queue; the token /
            # bias DMAs issue on the second HWDGE queue (ACT) so the SP
            # queue's ~0.65us-per-issue serialization doesn't delay the
            # early weight blocks.  The ACT issues all complete before the
            # first ACTIVATE needs that queue (mid-kernel ACT DMAs would
            # stall activations -- these are done by ~2us).  Each queue
            # still uses all 16 SDMA engines; wire bandwidth is shared.
            xt_sbs = []
            for j in range(EPC):
                xt_sb_j = const.tile([128, KH, CS[j]], DT, tag=f"xt{j}")
                xt_sbs.append(xt_sb_j)
            bz_sb = const.tile([128, EPC, 2, 8], f32)
            nc.scalar.dma_start(xt_sbs[0][:], xt_ds[0])
            nc.scalar.dma_start(bz_sb[:], bz_d)
            nc.scalar.dma_start(xt_sbs[1][:], xt_ds[1])
            cum = [128 * KH * CS[0] * 2]

            arrive = {}

            def track(key, nbytes):
                cum[0] += nbytes
                arrive[key] = SIM_DMA0 + cum[0] / SIM_RATE

            wgu_t, wd_t = {}, {}
            for e in range(EPC):
                for mp in range(MP):
                    # whole-block DMA: 4KB/partition contiguous = peak rate
                    t = wres.tile([128, 2, 2, KH, 128], WDT, tag=f"wg{e}_{mp}")
                    nc.sync.dma_start(t[:], wgu_d[e, mp])
                    track(("gu", e, mp), 128 * 4 * KH * 128 * WB)
                    wgu_t[(e, mp)] = t
                    if e == 0 and mp == 0:
                        cum[0] += 128 * EPC * 2 * 8 * 4   # bz shares the wire
                for hq in range(HQ):
                    t = wres.tile([128, 4, KF, 128], WDT, tag=f"wd{e}_{hq}")
                    nc.sync.dma_start(t[:], wd_d[e, hq])
                    track(("d", e, hq), 128 * 4 * KF * 128 * WB)
                    arrive[("d", e, hq, 0)] = arrive[("d", e, hq)]
                    arrive[("d", e, hq, 1)] = arrive[("d", e, hq)]
                    wd_t[(e, hq)] = t
                if e == 0:    # xt1's wire time lands amid e0's weights
                    cum[0] += 128 * KH * CS[1] * 2

            # ---- filler plan: simulate PE vs DMA arrivals ---------------
            units = []
            for e in range(EPC):
                for mp in range(MP):
                    units.append(("gu", e, mp))
                for hq in range(HQ):
                    for half in range(2):
                        units.append(("d", e, hq, half))
            fillers = {}
            t_pe = 900 + N_WARM * SIM_WARM_MM
            for ui, key in enumerate(units):
                mm_t = SIM_MM * CS[key[1]] / 144.0
                n_mm = 32 if key[0] == "gu" else 16
                gap = arrive[key] - t_pe
                n_fill = 0
                if gap > 200 and ui < len(units) - 2:
                    n_fill = min(10, int(0.9 * gap / SIM_FILL))
                fillers[key] = n_fill
                t_pe += n_fill * SIM_FILL
                t_pe = max(t_pe, arrive[key]) + n_mm * mm_t

            # ---- ACT table preload: tables load off the critical path
            # (reads uninitialized SBUF on purpose -- zero dependencies)
            wsink = const.tile([128, 8], f32)
            if not fast_act:
                nc.scalar.activation(wsink[:], wsink[:], AF.Identity)
            nc.scalar.activation(wsink[:], wsink[:], AF.Gelu_apprx_sigmoid)

            # ---- PE warm-up: dummy matmuls while the first blocks stream --
            # (wz memset runs on GpSimd, whose queue is free ~1.5us before
            # the DVE finishes its preamble -- warm-up starts that much
            # earlier)
            wz = const.tile([128, 512], DT)
            nc.gpsimd.memset(wz[:], 0.0)
            warm_ps = wpspool.tile([128, 512], f32)
            for i in range(N_WARM):
                nc.tensor.matmul(warm_ps[:], wz[:, 0:128], wz[:],
                                 start=(i == 0), stop=(i == N_WARM - 1))
            nc.vector.tensor_copy(wsink[:], warm_ps[:, 0:8])
            nc.sync.dma_start(dbg_d, wsink[:])

            # ---- main expert loop ---------------------------------------
            def pe_filler(n):
                # dummy matmuls pace the PE to the DMA stream so it never
                # idles long enough for the HAM clock-gate to re-throttle
                for _ in range(n):
                    nc.tensor.matmul(warm_ps[:], wz[:, 0:128], wz[:],
                                     start=True, stop=True)

            for e in range(EPC):
                C = CS[e]
                xt_sb = xt_sbs[e]
                gT = gpool.tile([128, KF, C], DT, tag="gT")
                for mp in range(MP):
                    wgut = wgu_t[(e, mp)]
                    for mi in range(2):
                        m = 2 * mp + mi
                        if mi == 0:
                            pe_filler(fillers[("gu", e, mp)])
                        psg = pspool.tile([128, C], f32, tag="ps")
                        psu = pspool.tile([128, C], f32, tag="ps")
                        for k in range(KH):
                            nc.tensor.matmul(psg[:], wgut[:, mi, 0, k],
                                             xt_sb[:, k],
                                             start=(k == 0), stop=(k == KH - 1))
                        for k in range(KH):
                            nc.tensor.matmul(psu[:], wgut[:, mi, 1, k],
                                             xt_sb[:, k],
                                             start=(k == 0), stop=(k == KH - 1))
                        if fast_act:
                            # glu = gelusig(psg + bg)      (clip provably dead)
                            glu = apool.tile([128, C], f32, tag="glu")
                            nc.scalar.activation(glu[:], psg[:],
                                                 AF.Gelu_apprx_sigmoid,
                                                 bias=bz_sb[:, e, 0, m:m + 1])
                            # up1/WS = (psu + (bu+1)) / WSCALE  (so g feeds
                            # the fp8e3(WSCALE*Wd) down matmul at true scale)
                            up = apool.tile([128, C], f32, tag="up")
                            nc.vector.tensor_scalar(up[:], psu[:],
                                                    bz_sb[:, e, 1, m:m + 1],
                                                    1.0 / WSCALE,
                                                    OP.add, OP.mult)
                        else:
                            # gate = min(psg + bg, 7)
                            gate = apool.tile([128, C], f32, tag="gate")
                            nc.vector.tensor_scalar(gate[:], psg[:],
                                                    bz_sb[:, e, 0, m:m + 1],
                                                    LIMIT, OP.add, OP.min)
                            # glu = gate * sigmoid(1.702 * gate)  (one ACT op)
                            glu = apool.tile([128, C], f32, tag="glu")
                            nc.scalar.activation(glu[:], gate[:],
                                                 AF.Gelu_apprx_sigmoid)
                            # (clip(psu + bu, -7, 7) + 1) / WSCALE  (bu has +1
                            #   folded pre-clip-shift: ACT adds bias + scales,
                            #   DVE clips in one op)
                            upb = apool.tile([128, C], f32, tag="upb")
                            nc.scalar.activation(upb[:], psu[:], AF.Identity,
                                                 bias=bz_sb[:, e, 1, m:m + 1])
                            upc = apool.tile([128, C], f32, tag="upc")
                            nc.vector.tensor_scalar(upc[:], upb[:],
                                                    LIMIT + 1.0,
                                                    -(LIMIT - 1.0),
                                                    OP.min, OP.max)
                            up = apool.tile([128, C], f32, tag="up")
                            nc.vector.tensor_scalar_mul(up[:], upc[:],
                                                        1.0 / WSCALE)
                        # gT[:, m] = up1 * glu   (cast to DT)
                        nc.vector.tensor_mul(out=gT[:, m], in0=up[:], in1=glu[:])
                yst = ypool.tile([128, MH, C], DT, tag="y")
                for hq in range(HQ):
                    wdt = wd_t[(e, hq)]
                    for hi in range(4):
                        h = 4 * hq + hi
                        if hi % 2 == 0:
                            pe_filler(fillers[("d", e, hq, hi // 2)])
                        psy = pspool.tile([128, C], f32, tag="ps")
                        for k in range(KF):
                            nc.tensor.matmul(psy[:], wdt[:, hi, k], gT[:, k],
                                             start=(k == 0), stop=(k == KF - 1))
                        nc.vector.tensor_copy(yst[:, h], psy[:])
                        if hi % 2 == 1:
                            h0 = h - 1
                            nc.sync.dma_start(yt_ds[e][h0 // 2],
                                              yst[:, h0:h0 + 2])
    nc.compile()
    return nc


def _get_nc(CS, fast_act):
    key = ("nc", CS, fast_act)
    if key not in _CACHE:
        _CACHE[key] = _build_nc(CS, fast_act)
    return _CACHE[key]


def _route(router_indices, routing_weights):
    """Per-expert unique token list + summed weights."""
    ri = np.asarray(router_indices)
    rw = np.asarray(routing_weights, dtype=np.float32)
    idxs, ws = [], []
    for e in range(E):
        m = ri == e
        any_m = m.any(axis=1)
        idx = np.nonzero(any_m)[0]
        w = (rw * m).sum(axis=1)[idx]
        idxs.append(idx.astype(np.int64))
        ws.append(w)
    return idxs, ws


def _fold_weights(gate_up_proj, gate_up_bias, down_proj, down_bias,
                  lora_gate_up_A, lora_gate_up_B, lora_down_A, lora_down_B,
                  perm):
    """LoRA-folded, gate/up-split, transfer-major packed per-core tensors.

    perm: [N_CORES][EPC] expert index assigned to each (core, slot).
    """
    w_dt = _np_w_dt()
    gup = np.asarray(gate_up_proj, dtype=np.float32)
    gub = np.asarray(gate_up_bias, dtype=np.float32)
    dwn = np.asarray(down_proj, dtype=np.float32)
    Agu = np.asarray(lora_gate_up_A, dtype=np.float32)
    Bgu = np.asarray(lora_gate_up_B, dtype=np.float32)
    Ad = np.asarray(lora_down_A, dtype=np.float32)
    Bd = np.asarray(lora_down_B, dtype=np.float32)

    # W_eff = W + A @ B * s    (batched over experts)
    wgu = gup + np.einsum("ehr,erd->ehd", Agu, Bgu) * SCALING     # [E, H, D]
    wdn = dwn + np.einsum("efr,erh->efh", Ad, Bd) * SCALING       # [E, F, H]

    wg = wgu[:, :, 0::2]                                          # [E, H, F]
    wu = wgu[:, :, 1::2]
    bgs = gub[:, 0::2]                                            # [E, F]
    bus = gub[:, 1::2] + 1.0                                      # fold (+1)

    def prep(w):
        # [E, K*128, M*128] -> [E, k, p, m, j] -> [E, p, m, k, j]
        w = w.reshape(E, KH, 128, MF, 128).transpose(0, 2, 3, 1, 4)
        return w
    wgp = prep(wg).reshape(E, 128, MP, 2, KH, 128)
    wup = prep(wu).reshape(E, 128, MP, 2, KH, 128)
    wgu_all = np.stack([wgp, wup], axis=4)  # [E, 128, MP, mi, gu, k, j]
    wdp = wdn.reshape(E, KF, 128, MH, 128).transpose(0, 2, 3, 1, 4)
    wdp = wdp.reshape(E, 128, HQ, 4, KF, 128)

    # biases: [E, 128, 2, 8]
    bz = np.stack([
        bgs.reshape(E, MF, 128).transpose(0, 2, 1),
        bus.reshape(E, MF, 128).transpose(0, 2, 1),
    ], axis=2)

    wgu_cores, wd_cores, bz_cores = [], [], []
    for c in range(N_CORES):
        sel = list(perm[c])
        wgu_cores.append(
            (wgu_all[sel].transpose(0, 2, 1, 3, 4, 5, 6)
             * WSCALE).astype(w_dt))
        wd_cores.append(
            (wdp[sel].transpose(0, 2, 1, 3, 4, 5) * WSCALE).astype(w_dt))
        bz_cores.append(np.ascontiguousarray(
            bz[sel].transpose(1, 0, 2, 3), dtype=np.float32))
    return {"wgu": wgu_cores, "wd": wd_cores, "bz": bz_cores,
            "wgu_eff": wgu}


def _expert_mlp_exact(x_e, Wg, Wu, bg, bu, Wd, bd):
    """fp32 numpy fallback (host) for capacity-overflow tokens."""
    gate = np.minimum(x_e @ Wg + bg, LIMIT)
    up = np.clip(x_e @ Wu + bu, -LIMIT, LIMIT)
    glu = gate / (1.0 + np.exp(-gate * ACT_ALPHA))
    g = (up + 1.0) * glu
    return g @ Wd + bd


def _host_expert(x, idx, e, gate_up_proj, gate_up_bias, down_proj, down_bias,
                 lora_gate_up_A, lora_gate_up_B, lora_down_A, lora_down_B):
    gup = np.asarray(gate_up_proj[e], dtype=np.float32)
    Agu = np.asarray(lora_gate_up_A[e], dtype=np.float32)
    Bgu = np.asarray(lora_gate_up_B[e], dtype=np.float32)
    wgu = gup + Agu @ Bgu * SCALING
    dwn = np.asarray(down_proj[e], dtype=np.float32)
    Ad = np.asarray(lora_down_A[e], dtype=np.float32)
    Bd = np.asarray(lora_down_B[e], dtype=np.float32)
    wdn = dwn + Ad @ Bd * SCALING
    gub = np.asarray(gate_up_bias[e], dtype=np.float32)
    return _expert_mlp_exact(x[idx], wgu[:, 0::2], wgu[:, 1::2],
                             gub[0::2], gub[1::2], wdn,
                             np.asarray(down_bias[e], dtype=np.float32))


def kernel(hidden_states, router_indices, routing_weights,
           gate_up_proj, gate_up_bias, down_proj, down_bias,
           lora_gate_up_A, lora_gate_up_B, lora_down_A, lora_down_B):
    from concourse import bass_utils

    np_dt = _np_dt()
    x = np.asarray(hidden_states, dtype=np.float32).reshape(T, H)
    idxs, ws = _route(router_indices, routing_weights)

    # sort experts by load (balances per-core totals); slot capacities track
    # the max load in each half so light slots do less dead work
    order = sorted(range(E), key=lambda e: -len(idxs[e]))
    perm = [[order[c], order[E - 1 - c]] for c in range(N_CORES)]
    C0 = min(C_MAX, max(8, -(-len(idxs[order[0]]) // 4) * 4))
    C1 = min(C0, max(8, -(-len(idxs[order[N_CORES]]) // 4) * 4))
    CS = (C0, C1)

    packed = _fold_weights(gate_up_proj, gate_up_bias, down_proj, down_bias,
                           lora_gate_up_A, lora_gate_up_B,
                           lora_down_A, lora_down_B, perm)

    # fast-act legality: the reference clips gate/up at +-7; if the actual
    # pre-activation magnitudes stay far below (host fp32 check on the real
    # routed tokens), the clips are dead code and the device can skip them.
    wgu_eff = packed["wgu_eff"]       # [E, H, D] fp32, LoRA folded
    gub = np.asarray(gate_up_bias, dtype=np.float32)
    gu_max = 0.0
    for e in range(E):
        if len(idxs[e]) == 0:
            continue
        gu = x[idxs[e][:C_MAX]] @ wgu_eff[e] + gub[e]
        gu_max = max(gu_max, float(np.abs(gu).max()))
    fast_act = gu_max < 6.0

    # gather + transpose tokens per expert: xt{j} [128, KH, CS[j]]
    in_maps = []
    for c in range(N_CORES):
        m = {"wgu": packed["wgu"][c], "wd": packed["wd"][c],
             "bz": packed["bz"][c]}
        for j in range(EPC):
            e = perm[c][j]
            Cj = CS[j]
            xt = np.zeros((128, KH, Cj), dtype=np_dt)
            idx = idxs[e][:Cj]
            if len(idx):
                # x[idx]/WSCALE : [n, H] -> T -> [KH, 128, n] -> [128, KH, n]
                xg = (x[idx].T.reshape(KH, 128, len(idx))
                      .transpose(1, 0, 2)) * (1.0 / WSCALE)
                xt[:, :, :len(idx)] = xg.astype(np_dt)
            m[f"xt{j}"] = xt
        in_maps.append(m)

    res = None
    try:
        nc = _get_nc(CS, fast_act)
        res = bass_utils.run_bass_kernel_spmd(
            nc, in_maps, core_ids=list(range(N_CORES)),
            **_CACHE.get("run_kwargs", {}))
    except Exception:
        try:
            nc = _get_nc(CS, fast_act)
            res = bass_utils.run_bass_kernel_spmd(
                nc, in_maps, core_ids=list(range(N_CORES)),
                **_CACHE.get("run_kwargs", {}))
        except Exception:
            res = None
    _CACHE["last_results"] = res
    if res is None:
        # device path failed: exact fp32 host fallback (slow but correct)
        out = np.zeros((T, H), dtype=np.float32)
        for e in range(E):
            idx = idxs[e]
            if not len(idx):
                continue
            y = _host_expert(x, idx, e, gate_up_proj, gate_up_bias,
                             down_proj, down_bias, lora_gate_up_A,
                             lora_gate_up_B, lora_down_A, lora_down_B)
            out[idx] += ws[e][:, None] * y
        return out.reshape(B_, S_, H)

    out = np.zeros((T, H), dtype=np.float32)
    for c in range(N_CORES):
        for j in range(EPC):
            e = perm[c][j]
            Cj = CS[j]
            yt = res.results[c][f"yt{j}"]      # [MH//2, 128, 2, Cj] fp16
            idx = idxs[e]
            n = min(len(idx), Cj)
            if n:
                # yt[hp, p, hi, t] -> y[t, (2hp+hi)*128+p]  (+ bias, host)
                y = np.ascontiguousarray(
                    yt[:, :, :, :n].transpose(3, 0, 2, 1)).reshape(n, H)
                y = y.astype(np.float32) + np.asarray(down_bias[e],
                                                      dtype=np.float32)
                out[idx[:n]] += ws[e][:n, None] * y
            if len(idx) > Cj:     # capacity overflow: exact host fallback
                ovf = idx[Cj:]
                y2 = _host_expert(x, ovf, e, gate_up_proj, gate_up_bias,
                                  down_proj, down_bias, lora_gate_up_A,
                                  lora_gate_up_B, lora_down_A, lora_down_B)
                out[ovf] += ws[e][Cj:, None] * y2
    return out.reshape(B_, S_, H)

